# revision 29
# baseline (speedup 1.0000x reference)
"""Trainium2 Bass kernel for EpisodicCuriosity (retrieval_knn).

Problem (per env): d2[b,m] = ||enc[b]-mem[m]||^2, take top-10 largest d2 per
query b, then a running-mean scan over the batch dim produces rewards (T,B).

Sharding: num_envs=64 split over 8 cores (8 envs/core), fully independent.

Host-side marshalling (inside kernel(), before dispatch): memory is
re-laid-out per env to feature-major (F, M) fp16 and augmented with two
extra contraction rows holding ||m||^2 split as fp16 hi + residual, so the
device GEMM directly produces mu[b,m] = ||m||^2 - 2*enc.mem. fp16 keeps
11 mantissa bits (tf32-class); measured output error ~5e-5 relative.

Per-core device pipeline (8 envs):
  - DMA fp16 memT tiles (f on partitions), 1 MB tiles.
  - mu = m2 - 2*enc.mem^T on PE: 4x (K=128,N=512) fp16 matmuls + 1x (K=2)
    for the m2 rows; per-env PSUM tiles (only the env's 32 rows are read).
  - mu is order-equivalent to d2 per row (d2 = relu(mu + e2[b])): top-10
    of 4096 per query via DVE max8 / match_replace / max8 on raw mu, then
    the affine+relu applied to just the (128,16) knn tile.
  - running-mean scan collapsed to a cumulative-sum matmul (block
    upper-triangular lhsT) + a handful of small elementwise ops.
"""

import numpy as np

import concourse.bacc as bacc
import concourse.bass as bass
import concourse.mybir as mybir
import concourse.tile as tile
from concourse import masks
from concourse.bass_utils import run_bass_kernel_spmd

# Problem constants (hardcoded per contract).
N_CORES = 8
NUM_ENVS = 64
E = NUM_ENVS // N_CORES  # envs per core = 8
B = 32
M = 4096
F = 512
KNN = 10
CLUSTER_DISTANCE = 0.008
EPS = 0.001
C = 0.01

f32 = mybir.dt.float32
f16 = mybir.dt.float16
AF = mybir.ActivationFunctionType
ALU = mybir.AluOpType
AX = mybir.AxisListType

MTILE = 512            # m per GEMM matmul (one PSUM bank)
JT = 1024              # m per DMA tile
NJ2 = M // JT          # 4 DMA tiles per env
NG = E // 4            # env groups of 4 (packed in 128 d2 partitions)
FA = F + 2             # feature rows + 2 rows of ||m||^2 (hi + residual)

EVICT_ENGINES = ("dve", "act")

_CACHE = {}


def _build():
    nc = bacc.Bacc("TRN2", target_bir_lowering=False, debug=False,
                   num_devices=N_CORES)
    enc_d = nc.dram_tensor("enc", [E, B, F], f32, kind="ExternalInput").ap()
    # memt[e, j2, p, (c, m')] = memT[e, 128c+p, JT*j2+m'] — each (e, j2) DMA
    # tile is one contiguous 8KB run per partition.
    mem_d = nc.dram_tensor("memt", [E, NJ2, 128, 4 * JT], f16,
                           kind="ExternalInput").ap()
    aux_d = nc.dram_tensor("aux", [E, 2, M], f16, kind="ExternalInput").ap()
    # consts: [:, :128] = block-diag upper-tri (lhsT of per-env cumsum),
    #         [:, 128]  = 1/(b+1) per (e,b) partition
    cst_d = nc.dram_tensor("cst", [128, 129], f32, kind="ExternalInput").ap()
    out_d = nc.dram_tensor("out", [NG, 128], f32, kind="ExternalOutput").ap()

    with tile.TileContext(nc) as tc:
        with (
            tc.tile_pool(name="const", bufs=1) as const_pool,
            tc.tile_pool(name="tmem", bufs=8) as t_pool,
            tc.tile_pool(name="taux", bufs=8) as aux_pool,
            tc.tile_pool(name="d2", bufs=2) as d2_pool,
            tc.tile_pool(name="small", bufs=4) as small_pool,
            tc.tile_pool(name="ps_mm", bufs=3, space="PSUM") as psum_mm,
            tc.tile_pool(name="ps_misc", bufs=2, space="PSUM") as psum_misc,
        ):
            # ---- constants ----
            eye = const_pool.tile([128, 128], f32)
            masks.make_identity(nc, eye[:])
            ones2 = const_pool.tile([2, 128], f16)
            nc.vector.memset(ones2[:], 1.0)
            negcd = const_pool.tile([128, 1], f32)
            nc.vector.memset(negcd[:], -CLUSTER_DISTANCE)

            def load_tiles(g, j2, el):
                e = 4 * g + el
                # memT tile: (128f, (c, m')) fp16, contiguous DMA
                tm = t_pool.tile([128, 4 * JT], f16, tag="tm")
                nc.sync.dma_start(tm[:], mem_d[e, j2])
                aux = aux_pool.tile([2, JT], f16, tag="aux")
                nc.scalar.dma_start(
                    aux[:], aux_d[e, :, j2 * JT:(j2 + 1) * JT])
                return tm, aux

            # tiny enc loads first so they aren't queued behind the 2MB
            # memory prefetches, then prefetch the first GEMM iteration
            enc_t_g = []
            for g in range(NG):
                enc_t = const_pool.tile([128, F], f32, tag=f"enc_{g}")
                src = enc_d[4 * g:4 * (g + 1)].rearrange("e b f -> (e b) f")
                nc.sync.dma_start(enc_t[:], src)
                enc_t_g.append(enc_t)
            preloaded = {(0, 0, el): load_tiles(0, 0, el) for el in range(4)}

            # cumsum/scan constants aren't needed until the first group's
            # top-k completes — load them after the prefetches
            cst = const_pool.tile([128, 129], f32)
            nc.sync.dma_start(cst[:], cst_d[:])
            tri = cst[:, 0:128]
            invn = cst[:, 128:129]

            # ---- enc prep (per group of 4 envs) ----
            e2_g = []
            encw_g = []  # [g][c] -> (128f, 128=(4e x 32b)) = -2*encT, fp16
            for g in range(NG):
                enc_t = enc_t_g[g]
                sq = const_pool.tile([128, F], f32, tag="encsq")
                e2 = const_pool.tile([128, 1], f32, tag=f"e2_{g}")
                nc.scalar.activation(sq[:], enc_t[:], AF.Square,
                                     accum_out=e2[:])
                e2_g.append(e2)
                row = []
                for c in range(4):
                    ps = psum_misc.tile([128, 128], f32, tag="psmisc")
                    nc.tensor.transpose(ps[:], enc_t[:, 128 * c:128 * (c + 1)],
                                        eye[:])
                    w = const_pool.tile([128, 128], f16, tag=f"encw_{g}_{c}")
                    nc.scalar.mul(w[:], ps[:], -2.0)
                    row.append(w)
                encw_g.append(row)

            # ---- main loop ----
            for g in range(NG):
                d2 = d2_pool.tile([128, M], f32)
                cand = small_pool.tile([128, 128], f32, tag="cand")
                for j2 in range(NJ2):
                    for el in range(4):
                        tm, aux = preloaded.pop((g, j2, el), (None, None))
                        if tm is None:
                            tm, aux = load_tiles(g, j2, el)

                        ps_mm = psum_mm.tile([128, JT], f32, tag="psmm")
                        for h in range(2):
                            pslice = ps_mm[:, MTILE * h:MTILE * (h + 1)]
                            for c in range(4):
                                nc.tensor.matmul(
                                    pslice, lhsT=encw_g[g][c][:],
                                    rhs=tm[:, JT * c + MTILE * h:
                                           JT * c + MTILE * (h + 1)],
                                    start=(c == 0), stop=False)
                            nc.tensor.matmul(
                                pslice, lhsT=ones2[:],
                                rhs=aux[:, MTILE * h:MTILE * (h + 1)],
                                start=False, stop=True)

                        # evict this env's 32 rows of mu into d2, halves
                        # split over ACT+DVE for latency
                        dst = d2[32 * el:32 * (el + 1),
                                 j2 * JT:(j2 + 1) * JT]
                        srcp = ps_mm[32 * el:32 * (el + 1), :]
                        nc.scalar.copy(dst[:, 0:MTILE], srcp[:, 0:MTILE])
                        nc.vector.tensor_copy(dst[:, MTILE:JT],
                                              srcp[:, MTILE:JT])

                    # streaming top-16 per 512-wide octant: fully hidden
                    # behind the GEMM; final selection is on (128, 128)
                    for oh in range(2):
                        o = 2 * j2 + oh
                        oct_ = d2[:, o * MTILE:(o + 1) * MTILE]
                        cnd = cand[:, 16 * o:16 * o + 16]
                        nc.vector.max(cnd[:, 0:8], oct_)
                        nc.vector.match_replace(oct_, cnd[:, 0:8], oct_,
                                                -1e30)
                        nc.vector.max(cnd[:, 8:16], oct_)

                # ---- top-10 of the 128 octant candidates per query ----
                knn = small_pool.tile([128, 16], f32, tag="knn")
                nc.vector.max(knn[:, 0:8], cand[:])
                nc.vector.match_replace(cand[:], knn[:, 0:8], cand[:], -1e30)
                nc.vector.max(knn[:, 8:16], cand[:])
                # d2 = relu(mu + e2) applied to the 16 survivors only
                knn2 = small_pool.tile([128, 16], f32, tag="knn2")
                nc.scalar.activation(knn2[:], knn[:], AF.Relu,
                                     bias=e2_g[g][:], scale=1.0)
                kt = knn2[:, 0:KNN]

                # ---- scan: cumsum via block-triangular matmul ----
                ps_c = psum_misc.tile([128, KNN], f32, tag="psmisc")
                nc.tensor.matmul(ps_c[:], lhsT=tri, rhs=kt, start=True,
                                 stop=True)
                rm = small_pool.tile([128, KNN], f32, tag="rm")
                nc.vector.tensor_scalar_mul(rm[:], ps_c[:], invn)
                rcp = small_pool.tile([128, KNN], f32, tag="rcp")
                nc.vector.reciprocal(rcp[:], rm[:])
                q = small_pool.tile([128, KNN], f32, tag="q")
                nc.vector.tensor_tensor(q[:], kt, rcp[:], op=ALU.mult)
                t1 = small_pool.tile([128, KNN], f32, tag="t1")
                nc.scalar.activation(t1[:], q[:], AF.Relu, bias=negcd[:])
                t2 = small_pool.tile([128, KNN], f32, tag="t2")
                nc.vector.tensor_scalar_add(t2[:], t1[:], EPS)
                r = small_pool.tile([128, KNN], f32, tag="r")
                nc.vector.reciprocal(r[:], t2[:])
                s = small_pool.tile([128, 1], f32, tag="s")
                nc.vector.reduce_sum(s[:], r[:], axis=AX.X)
                sim = small_pool.tile([128, 1], f32, tag="sim")
                nc.scalar.activation(sim[:], s[:], AF.Sqrt, scale=EPS)
                simc = small_pool.tile([128, 1], f32, tag="simc")
                nc.vector.tensor_scalar_add(simc[:], sim[:], C)
                rew = small_pool.tile([128, 1], f32, tag="rew")
                nc.vector.reciprocal(rew[:], simc[:])
                nc.scalar.dma_start(out_d[g:g + 1, :], rew[:])

    nc.compile()
    return nc


def _consts():
    blk = np.triu(np.ones((B, B), dtype=np.float32))  # lhsT[i,b] = i<=b
    tri = np.zeros((128, 128), dtype=np.float32)
    for e in range(4):
        tri[e * B:(e + 1) * B, e * B:(e + 1) * B] = blk
    invn = np.tile((1.0 / np.arange(1, B + 1, dtype=np.float32)), 4)
    cst = np.zeros((128, 129), dtype=np.float32)
    cst[:, :128] = tri
    cst[:, 128] = invn
    return cst


def _marshal_memory(mem):
    """(n, M, F) fp32 -> memt (n, NJ2, 128, 4*JT) fp16 feature-major tiles
    (contiguous per partition) + aux (n, 2, M) fp16 rows of ||m||^2
    (value + residual)."""
    n = mem.shape[0]
    # memT[e, f, m] -> [e, j2, p, c, m'] with f = 128c+p, m = JT*j2+m'
    mt = mem.swapaxes(1, 2).astype(np.float16)          # (n, F, M)
    mt = mt.reshape(n, 4, 128, NJ2, JT)                  # (n, c, p, j2, m')
    memt = np.ascontiguousarray(mt.transpose(0, 3, 2, 1, 4)).reshape(
        n, NJ2, 128, 4 * JT)
    m2 = np.einsum("nmf,nmf->nm", mem, mem, dtype=np.float32,
                   optimize=True).astype(np.float32)
    aux = np.empty((n, 2, M), dtype=np.float16)
    hi = m2.astype(np.float16)
    lo = (m2 - hi.astype(np.float32)).astype(np.float16)
    aux[:, 0, :] = hi
    aux[:, 1, :] = lo
    return memt, aux


def run_kernel(encoded_states, memory, trace=False):
    if "nc" not in _CACHE:
        _CACHE["nc"] = _build()
    nc = _CACHE["nc"]
    cst = _consts()
    enc = np.ascontiguousarray(encoded_states, dtype=np.float32)
    mem = np.ascontiguousarray(memory, dtype=np.float32)
    memt, aux = _marshal_memory(mem)
    in_maps = [
        {"enc": enc[i * E:(i + 1) * E], "memt": memt[i * E:(i + 1) * E],
         "aux": aux[i * E:(i + 1) * E], "cst": cst}
        for i in range(N_CORES)
    ]
    res = run_bass_kernel_spmd(nc, in_maps, list(range(N_CORES)), trace=trace)
    outs = []
    for i in range(N_CORES):
        o = np.asarray(res.results[i]["out"])  # (NG, 128)
        outs.append(o.reshape(E, B))
    full = np.concatenate(outs, axis=0).astype(np.float32)
    return full, res


def kernel(encoded_states, memory):
    full, _ = run_kernel(encoded_states, memory)
    return full
